# revision 29
# baseline (speedup 1.0000x reference)
"""CapsNet feature extractor on 8 Trainium2 NeuronCores (Bass/Tile).

Sharding: phase A (conv1 -> bn/relu -> pconv -> squash) is data-parallel over
batch (8 images/core). An AllToAll reshards u from batch-sharded to
routes-sharded (576 routes/core = 32 pconv output channels/core, since
route r = oc*18 + q under the raw row-major reshape). Phase B (u_hat, dynamic
routing) is routes-sharded with an AllReduce on s each iteration; the final
FC stack is computed redundantly on every core; core 0's output is returned.

v2: h is stored phase-split (4 polyphase sub-images) so the stride-2 pconv
reads contiguous rows; the u reshard uses a contiguous A2A payload + hardware
DMA-transpose into the (c, route-group) layout; u_hat is staged r-major
through DRAM with batched 320B-run DMAs instead of per-route scatters.
"""
import os
import sys
sys.path.insert(0, '/opt/trn_rl_repo')
import numpy as np
from contextlib import ExitStack

import concourse.bass as bass
import concourse.bacc as bacc
import concourse.mybir as mybir
from concourse import tile
from concourse.bass_utils import run_bass_kernel_spmd

dt = mybir.dt
AF = mybir.ActivationFunctionType
ALU = mybir.AluOpType
AX = mybir.AxisListType

N_CORES = 8
B = 64; IN_C = 3; IMG = 64
NCAP = 8; PC = 32; ND = 10; DC = 16; FEAT = 128
C1 = 56
PR = 24
ROUTES = 4608
BPC = B // N_CORES
RPC = ROUTES // N_CORES          # 576
NO = ND * DC                     # 160
NG = RPC // 4                    # 144 groups of 4 routes (one per q4 block)
NT = 5                           # u_hat partition tiles (4 x 128 + 1 x 64)
F32, BF16, F32R = dt.float32, dt.bfloat16, dt.float32r


def ap(t, offset, dims):
    """Manual access pattern; dims = [[step, count], ...] outer->inner, elems."""
    if isinstance(t, bass.AP):
        return bass.AP(tensor=t.tensor, offset=t.offset + offset,
                       ap=[list(d) for d in dims])
    return bass.AP(tensor=t, offset=offset, ap=[list(d) for d in dims])


def fap(tl, p0, pcnt, off, dims, pstep_mult=1):
    """AP into an SBUF tile AP `tl`: partition run [p0, p0+pcnt) with partition
    step `pstep_mult` rows, then free dims (offset `off` within partition)."""
    pstep = tl.ap[0][0]
    return bass.AP(tensor=tl.tensor, offset=tl.offset + p0*pstep + off,
                   ap=[[pstep*pstep_mult, pcnt]] + [list(d) for d in dims])


def _split9(pbase):
    if pbase + 9 <= 128:
        return [(0, pbase, 0, 9)]
    if pbase >= 128:
        return [(1, pbase - 128, 0, 9)]
    c = 128 - pbase
    return [(0, pbase, 0, c), (1, 0, c, 9 - c)]


def build(debug=False, declare_dbg=None):
    if declare_dbg is None:
        declare_dbg = debug
    nc = bacc.Bacc("TRN2", target_bir_lowering=False, debug=False,
                   num_devices=N_CORES)

    xs = nc.declare_dram_parameter("xs", [BPC, IN_C, IMG, IMG], F32R, isOutput=False)
    w1 = nc.declare_dram_parameter("w1", [243, 256], F32R, isOutput=False)
    b1 = nc.declare_dram_parameter("b1", [256], F32, isOutput=False)
    w2 = nc.declare_dram_parameter("w2", [128, 2*81*2*128], BF16, isOutput=False)
    b2 = nc.declare_dram_parameter("b2", [256], F32, isOutput=False)
    wk = nc.declare_dram_parameter("wk", [128, NG*NO], BF16, isOutput=False)
    fc1t = nc.declare_dram_parameter("fc1t", [161, 512], F32R, isOutput=False)
    fc2t = nc.declare_dram_parameter("fc2t", [513, 256], F32R, isOutput=False)
    fc3t = nc.declare_dram_parameter("fc3t", [257, 128], F32R, isOutput=False)
    ident = nc.declare_dram_parameter("ident", [128, 128], F32R, isOutput=False)
    onesd = nc.declare_dram_parameter("onesd", [128, 64], F32R, isOutput=False)
    out = nc.declare_dram_parameter("out", [B, FEAT], F32, isOutput=True)
    dbg = {}
    if declare_dbg:
        dbg['u_dbg'] = nc.declare_dram_parameter("u_dbg", [BPC, ROUTES, PC], BF16, isOutput=True)
        dbg['ua_dbg'] = nc.declare_dram_parameter("ua_dbg", [128, NG*B], BF16, isOutput=True)
        dbg['uh_dbg'] = nc.declare_dram_parameter("uh_dbg", [NT, 128, B*NO], BF16, isOutput=True)

    # A2A payload per dst core: [img(8), chb(8), q(18), q4(4), c(32)] bf16
    u_send = nc.dram_tensor("u_send", [N_CORES, BPC*RPC*PC], BF16)
    u_recv = nc.dram_tensor("u_recv", [N_CORES, BPC*RPC*PC], BF16)
    # u_hat staged r-major: [r(576), b(64), no(160)] bf16
    uh_dram = nc.dram_tensor("uh_dram", [RPC, B*NO], BF16)
    s_send = nc.dram_tensor("s_send", [3, B*NO], F32)
    s_recv = nc.dram_tensor("s_recv", [3, B*NO], F32)
    v_stage = nc.dram_tensor("v_stage", [ND, B*DC], F32R)
    groups = [list(range(N_CORES))]

    with tile.TileContext(nc) as tc, ExitStack() as top:
        consts = top.enter_context(tc.tile_pool(name="consts", bufs=1))

        b1t = consts.tile([128, 2], F32)
        nc.sync.dma_start(b1t[:], ap(b1, 0, [[1, 128], [128, 2]]))
        b2t = consts.tile([128, 2], F32)
        nc.sync.dma_start(b2t[:], ap(b2, 0, [[1, 128], [128, 2]]))
        onesb = consts.tile([128, 1], BF16)
        nc.vector.memset(onesb[:], 1.0)
        onesrow = consts.tile([1, 128], BF16)
        nc.vector.memset(onesrow[:], 1.0)
        onesf = consts.tile([128, 64], F32R)
        nc.sync.dma_start(onesf[:], onesd[:])
        identT = consts.tile([128, 128], F32R)
        nc.sync.dma_start(identT[:], ident[:])
        zt = consts.tile([128, 1], F32)
        nc.vector.memset(zt[:], 0.0)
        epst = consts.tile([128, 1], F32)
        nc.vector.memset(epst[:], 1e-8)

        # ============ PHASE A ============
        with tc.tile_pool(name="pha", bufs=1) as pha, \
             tc.tile_pool(name="img", bufs=2) as ipool, \
             tc.tile_pool(name="psumA", bufs=2, space="PSUM") as psumA:
            w1a = pha.tile([128, 256], F32R)
            nc.sync.dma_start(w1a[:], w1[0:128, :])
            w1b = pha.tile([128, 256], F32R)
            nc.sync.dma_start(w1b[0:115, :], w1[128:243, :])
            w2sb = pha.tile([128, 2*81*2*128], BF16)
            nc.sync.dma_start(w2sb[:], w2[:])

            for img in range(BPC):
                xa = ipool.tile([128, C1*C1], F32R, tag="xa")
                xb = ipool.tile([128, C1*C1], F32R, tag="xb")
                for ic in range(IN_C):
                    for ky in range(9):
                        pbase = ic*81 + ky*9
                        soff = img*IN_C*IMG*IMG + ic*IMG*IMG + ky*IMG
                        for (tdst, p0, kx0, cnt) in _split9(pbase):
                            dstt = xa if tdst == 0 else xb
                            nc.sync.dma_start(
                                dstt[p0:p0+cnt, :],
                                ap(xs, soff + kx0, [[1, cnt], [IMG, C1], [1, C1]]))

                # h phase-split: [128, ichg(2) x phase(4) x 784] bf16
                # phase = (y%2)*2 + (x%2); sub-image pos (y//2, x//2) in 28x28
                h = ipool.tile([128, 2*C1*C1], BF16, tag="h")
                for oyc in range(7):
                    for och in range(2):
                        ps = psumA.tile([128, 448], F32, tag="psc1")
                        nc.tensor.matmul(ps[:], w1a[:, och*128:(och+1)*128],
                                         xa[:, oyc*448:(oyc+1)*448],
                                         start=True, stop=False)
                        nc.tensor.matmul(ps[:], w1b[0:115, och*128:(och+1)*128],
                                         xb[0:115, oyc*448:(oyc+1)*448],
                                         start=False, stop=True)
                        for py in range(2):
                            for px in range(2):
                                nc.scalar.activation(
                                    fap(h, 0, 128,
                                        och*3136 + (py*2+px)*784 + oyc*112,
                                        [[28, 4], [1, 28]]),
                                    fap(ps, 0, 128, py*56 + px,
                                        [[112, 4], [2, 28]]),
                                    AF.Relu, bias=b1t[:, och:och+1])

                # pconv: accumulate 81 taps from the 4 phase sub-images
                pst = ipool.tile([128, 2*PR*PR], F32, tag="pst")
                for och in range(2):
                    psA = psumA.tile([128, 288], F32, tag="psA")
                    psB = psumA.tile([128, 288], F32, tag="psB")
                    for ich in range(2):
                        for ky in range(9):
                            for kx in range(9):
                                widx = ((ich*81 + ky*9 + kx)*2 + och)*128
                                lhsT = w2sb[:, widx:widx+128]
                                first = (ich == 0 and ky == 0 and kx == 0)
                                last = (ich == 1 and ky == 8 and kx == 8)
                                base = (ich*3136 + ((ky % 2)*2 + (kx % 2))*784
                                        + (ky//2)*28 + (kx//2))
                                nc.tensor.matmul(
                                    psA[:], lhsT,
                                    fap(h, 0, 128, base, [[28, 12], [1, 24]]),
                                    start=first, stop=last)
                                nc.tensor.matmul(
                                    psB[:], lhsT,
                                    fap(h, 0, 128, base + 12*28, [[28, 12], [1, 24]]),
                                    start=first, stop=last)
                    nc.scalar.activation(pst[:, och*576:och*576+288], psA[:],
                                         AF.Identity, bias=b2t[:, och:och+1])
                    nc.scalar.activation(pst[:, och*576+288:och*576+576], psB[:],
                                         AF.Identity, bias=b2t[:, och:och+1])

                p2 = ipool.tile([128, 2*PR*PR], F32, tag="p2")
                nc.vector.tensor_mul(p2[:], pst[:], pst[:])
                sn = ipool.tile([128, 36], F32, tag="sn")
                nc.vector.tensor_reduce(sn[:], fap(p2, 0, 128, 0, [[32, 36], [1, 32]]),
                                        AX.X, ALU.add)
                sn1 = ipool.tile([128, 36], F32, tag="sn1")
                nc.vector.tensor_scalar_add(sn1[:], sn[:], 1.0)
                rde = ipool.tile([128, 36], F32, tag="rde")
                nc.vector.reciprocal(rde[:], sn1[:])
                sqr = ipool.tile([128, 36], F32, tag="sqr")
                nc.scalar.activation(sqr[:], sn[:], AF.Sqrt, bias=epst[:, :])
                rsq = ipool.tile([128, 36], F32, tag="rsq")
                nc.vector.reciprocal(rsq[:], sqr[:])
                scl = ipool.tile([128, 36], F32, tag="scl")
                nc.vector.tensor_mul(scl[:], sn[:], rde[:])
                nc.vector.tensor_mul(scl[:], scl[:], rsq[:])
                usq = ipool.tile([128, 2*PR*PR], BF16, tag="usq")
                nc.vector.tensor_mul(usq[:], pst[:],
                                     fap(scl, 0, 128, 0, [[1, 36], [0, 32]]))
                if debug:
                    for och in range(2):
                        nc.sync.dma_start(
                            ap(dbg['u_dbg'], img*ROUTES*PC + och*128*576,
                               [[576, 128], [1, 576]]),
                            usq[:, och*576:(och+1)*576])
                # send: dst core j = och*4+kq gets channels 32j..32j+32.
                # payload layout [img, chb(8), q(18), q4(4), c(32)]:
                #   addr = ((img*8+chb)*18+q)*128 + q4*32 + c,  ch_loc = 8*q4+chb
                for kq in range(4):
                    for och in range(2):
                        j = och*4 + kq
                        nc.sync.dma_start(
                            ap(u_send, j*BPC*RPC*PC + img*RPC*PC,
                               [[32, 4], [18*128, 8], [128, 18], [1, 32]]),
                            fap(usq, 32*kq, 32, och*576, [[32, 18], [1, 32]]))

        nc.gpsimd.collective_compute("AllToAll", ALU.bypass, replica_groups=groups,
                                     ins=[u_send[:]], outs=[u_recv[:]])

        # ============ PHASE B: u_hat ============
        uhp = top.enter_context(tc.tile_pool(name="uhp", bufs=1))
        uh = [uhp.tile([128, B*NO], BF16, name=f"uh{t}", tag=f"uh{t}")
              for t in range(NT)]
        nc.vector.memset(uh[4][64:128, :], 0.0)

        with tc.tile_pool(name="utp", bufs=1) as utp, \
             tc.tile_pool(name="wstr", bufs=2) as wstr, \
             tc.tile_pool(name="stgp", bufs=2) as stgp, \
             tc.tile_pool(name="psumB", bufs=2, space="PSUM") as psumB:
            # u_all128[p=(q4,c), col=b*144+g] via HW dma-transpose per src:
            # in [1152 rows=(img,chb,q), 128 cols=(q4,c)] -> out [128, 1152]
            u_all = utp.tile([128, NG*B], BF16)
            for src in range(N_CORES):
                nc.sync.dma_start(
                    u_all[:, src*1152:(src+1)*1152],
                    ap(u_recv, src*BPC*RPC*PC, [[128, 1152], [1, 128]]),
                    transpose=True)

            if debug:
                nc.sync.dma_start(dbg['ua_dbg'][:], u_all[:])
            tc.strict_bb_all_engine_barrier()
            # r = q4*144 + g ; lhsT = u_all[32q4:32q4+32, cols b*144+g] (64 b)
            # rhs = wchunk[32q4:32q4+32, (g%24)*160 : +160]
            GCH = 24                       # g-groups per W chunk
            for ck in range(NG // GCH):    # 6 W chunks, each reused by 4 q4
                wchunk = wstr.tile([128, GCH*NO], BF16, tag="wchunk")
                nc.sync.dma_start(wchunk[:], wk[:, ck*GCH*NO:(ck+1)*GCH*NO])
                # interleave the 4 q4 tile-positions so LDWEIGHTS/MATMUL of
                # different PE subarray rows overlap; evicts alternate ACT/DVE
                stgts = [stgp.tile([64, GCH*NO], BF16, tag=f"stgt{q4}",
                                   name=f"stgt{ck}_{q4}")
                         for q4 in range(4)]
                for i3 in range(GCH // 3):
                    psrs = [psumB.tile([64, 3*NO], F32, tag=f"psr{q4}",
                                       name=f"psr{ck}_{i3}_{q4}")
                            for q4 in range(4)]
                    for j in range(3):
                        i = i3*3 + j
                        g = ck*GCH + i
                        for q4 in range(4):
                            nc.tensor.matmul(
                                psrs[q4][:, j*NO:(j+1)*NO],
                                fap(u_all, 32*q4, 32, g, [[NG, B]]),
                                wchunk[32*q4:32*q4+32, i*NO:(i+1)*NO],
                                start=True, stop=True, tile_position=(32*q4, 0))
                    for q4 in range(4):
                        dst = stgts[q4][:, i3*3*NO:(i3+1)*3*NO]
                        if q4 % 2 == 0:
                            nc.scalar.activation(dst, psrs[q4][:],
                                                 AF.Identity, bias=zt[0:64, :])
                        else:
                            nc.vector.tensor_copy(dst, psrs[q4][:])
                for q4 in range(4):
                    rb = q4*NG + ck*GCH
                    nc.sync.dma_start(
                        ap(uh_dram, rb*B*NO,
                           [[NO, B], [B*NO, GCH], [1, NO]]),
                        stgts[q4][:])
            tc.strict_bb_all_engine_barrier()
            for t in range(NT):
                rows = 128 if t < 4 else 64
                nc.sync.dma_start(
                    uh[t][0:rows, :],
                    ap(uh_dram, t*128*B*NO, [[B*NO, rows], [1, B*NO]]))
                if debug:
                    nc.sync.dma_start(
                        ap(dbg['uh_dbg'], t*128*B*NO, [[B*NO, rows], [1, B*NO]]),
                        uh[t][0:rows, :])

        # ============ routing ============
        rp = top.enter_context(tc.tile_pool(name="rp", bufs=1))
        logits = rp.tile([128, NT*B*ND], F32)
        nc.vector.memset(logits[:], 0.0)
        c_t = rp.tile([128, NT*B*ND], BF16)
        s_sb = rp.tile([10, B*DC], F32)
        v_sb = rp.tile([10, B*DC], F32)
        vb16 = rp.tile([10, B*DC], BF16)
        sq2 = rp.tile([10, B*DC], F32)
        snv = rp.tile([10, B], F32)
        snv1 = rp.tile([10, B], F32)
        rdev = rp.tile([10, B], F32)
        sqv = rp.tile([10, B], F32)
        rsqv = rp.tile([10, B], F32)
        sclv = rp.tile([10, B], F32)
        sume = rp.tile([128, NT*B], F32)
        rece = rp.tile([128, NT*B], F32)

        with tc.tile_pool(name="agg", bufs=1) as agg, \
             tc.tile_pool(name="m2cp", bufs=3) as m2cp, \
             tc.tile_pool(name="sstg", bufs=2) as sstg, \
             tc.tile_pool(name="psumR", bufs=4, space="PSUM") as psumR:
            m2bs = [agg.tile([128, B*NO//2], BF16, name=f"m2b{i}") for i in range(2)]
            red = agg.tile([128, B*ND], F32)
            v_rep = agg.tile([128, B*NO], BF16)
            vrow = agg.tile([1, B*NO], BF16)

            for it in range(3):
                # ---- partial s = sum over local routes of c * u_hat ----
                for chk in range(22):
                    b0 = chk*3
                    nb = min(3, B - b0)
                    w = nb*NO
                    pss = psumR.tile([128, 512], F32, tag="pss")
                    for t in range(NT):
                        kk = 128 if t < 4 else 64
                        if it == 0:
                            rhs = uh[t][0:kk, b0*NO:b0*NO + w]
                        else:
                            m2c = m2cp.tile([128, 512], BF16, tag="m2c")
                            nc.vector.tensor_mul(
                                m2c[0:kk, 0:w], uh[t][0:kk, b0*NO:b0*NO + w],
                                fap(c_t, 0, kk, t*B*ND + b0*ND,
                                    [[ND, nb], [1, ND], [0, DC]]))
                            rhs = m2c[0:kk, 0:w]
                        nc.tensor.matmul(pss[0:1, 0:w], onesb[0:kk, :], rhs,
                                         start=(t == 0), stop=(t == 4))
                    sst = sstg.tile([1, 512], F32, tag="sst")
                    nc.scalar.activation(sst[0:1, 0:w], pss[0:1, 0:w],
                                         AF.Identity, bias=zt[0:1, :],
                                         scale=(0.1 if it == 0 else 1.0))
                    nc.sync.dma_start(ap(s_send, it*B*NO + b0*NO, [[1, w]]),
                                      sst[0:1, 0:w])
                nc.gpsimd.collective_compute(
                    "AllReduce", ALU.add, replica_groups=groups,
                    ins=[ap(s_send, it*B*NO, [[1, B*NO]])],
                    outs=[ap(s_recv, it*B*NO, [[1, B*NO]])])

                # ---- v = squash(s) in [10p=n, (b, o)] ----
                nc.sync.dma_start(s_sb[:],
                                  ap(s_recv, it*B*NO, [[DC, ND], [NO, B], [1, DC]]))
                nc.vector.tensor_mul(sq2[:], s_sb[:], s_sb[:])
                nc.vector.tensor_reduce(snv[:], fap(sq2, 0, 10, 0, [[DC, B], [1, DC]]),
                                        AX.X, ALU.add)
                nc.vector.tensor_scalar_add(snv1[:], snv[:], 1.0)
                nc.vector.reciprocal(rdev[:], snv1[:])
                nc.scalar.activation(sqv[:], snv[:], AF.Sqrt, bias=epst[0:10, :])
                nc.vector.reciprocal(rsqv[:], sqv[:])
                nc.vector.tensor_mul(sclv[:], snv[:], rdev[:])
                nc.vector.tensor_mul(sclv[:], sclv[:], rsqv[:])
                nc.vector.tensor_mul(v_sb[:], s_sb[:],
                                     fap(sclv, 0, 10, 0, [[1, B], [0, DC]]))
                if it == 2:
                    nc.sync.dma_start(v_stage[:].bitcast(F32), v_sb[:])
                    break
                nc.vector.tensor_copy(vb16[:], v_sb[:])
                # vrow [1, (b,n,o)] then broadcast to 128 partitions via PE
                for n in range(ND):
                    nc.sync.dma_start(
                        fap(vrow, 0, 1, n*DC, [[NO, B], [1, DC]]),
                        vb16[n:n+1, :])
                for k in range(B*NO//512):
                    psv = psumR.tile([128, 512], F32, tag="psv")
                    nc.tensor.matmul(psv[:], onesrow[0:1, :],
                                     vrow[0:1, k*512:(k+1)*512],
                                     start=True, stop=True)
                    nc.scalar.activation(v_rep[:, k*512:(k+1)*512], psv[:],
                                         AF.Identity, bias=zt[:, :])

                # ---- agreement: logits += sum_o u_hat * v (half-tiles) ----
                # muls on GpSimd, reduces on DVE -> the two pipeline
                for t in range(NT):
                    for hh in range(2):
                        sl = slice(hh*B*NO//2, (hh+1)*B*NO//2)
                        mb = m2bs[(t*2 + hh) % 2]
                        nc.gpsimd.tensor_mul(mb[:], uh[t][:, sl], v_rep[:, sl])
                        nc.vector.tensor_reduce(
                            red[:, hh*B*ND//2:(hh+1)*B*ND//2],
                            fap(mb, 0, 128, 0, [[DC, B*ND//2], [1, DC]]),
                            AX.X, ALU.add)
                    nc.vector.tensor_add(
                        logits[:, t*B*ND:(t+1)*B*ND],
                        logits[:, t*B*ND:(t+1)*B*ND],
                        red[:])

                # ---- c = softmax(logits) over n (no max-subtraction) ----
                eexp = v_rep[:].bitcast(F32)        # [128, 5120] f32 view
                nc.scalar.activation(eexp[:, 0:NT*B*ND], logits[:], AF.Exp, bias=zt[:, :])
                nc.vector.tensor_reduce(
                    sume[:], fap(eexp, 0, 128, 0, [[ND, NT*B], [1, ND]]),
                    AX.X, ALU.add)
                nc.vector.reciprocal(rece[:], sume[:])
                nc.vector.tensor_mul(c_t[:], eexp[:, 0:NT*B*ND],
                                     fap(rece, 0, 128, 0, [[1, NT*B], [0, ND]]))

        # ============ FC head (redundant on every core) ============
        with tc.tile_pool(name="fcp", bufs=1) as fcp, \
             tc.tile_pool(name="psumF", bufs=1, space="PSUM") as psumF:
            fta = fcp.tile([128, B], F32R)
            ftb = fcp.tile([128, B], F32R)
            for n in range(ND):
                dstt, p0 = (fta, n*DC) if n < 8 else (ftb, (n-8)*DC)
                nc.sync.dma_start(dstt[p0:p0+DC, :],
                                  ap(v_stage, n*B*DC, [[1, DC], [DC, B]]))
            nc.sync.dma_start(ftb[32:33, :], onesd[0:1, :])

            fc1a = fcp.tile([128, 512], F32R)
            nc.sync.dma_start(fc1a[:], fc1t[0:128, :])
            fc1b = fcp.tile([128, 512], F32R)
            nc.sync.dma_start(fc1b[0:33, :], fc1t[128:161, :])
            pf1 = psumF.tile([64, 512], F32, tag="pf1")
            nc.tensor.matmul(pf1[:], fta[:, 0:64],
                             fc1a[:], start=True, stop=False)
            nc.tensor.matmul(pf1[:], ftb[0:33, 0:64],
                             fc1b[0:33, :], start=False, stop=True)
            f1 = fcp.tile([64, 512], F32R)
            nc.scalar.activation(f1[:], pf1[:], AF.Relu, bias=zt[0:64, :])

            f1T = fcp.tile([128, 4*64], F32R)
            for k in range(4):
                ptr = psumF.tile([128, 64], F32R, tag="ptr", bufs=2)
                nc.tensor.transpose(ptr[:], f1[:, k*128:(k+1)*128], identT[0:64, 0:64])
                nc.scalar.activation(f1T[:, k*64:(k+1)*64], ptr[:], AF.Identity, bias=zt[:, :])

            fc2a = fcp.tile([128, 4*256], F32R)
            nc.sync.dma_start(fc2a[:], ap(fc2t, 0, [[256, 128], [128*256, 4], [1, 256]]))
            fc2b = fcp.tile([1, 256], F32R)
            nc.sync.dma_start(fc2b[:], fc2t[512:513, :])
            pf2 = psumF.tile([64, 256], F32, tag="pf2")
            for k in range(4):
                nc.tensor.matmul(pf2[:], f1T[:, k*64:(k+1)*64],
                                 fc2a[:, k*256:(k+1)*256],
                                 start=(k == 0), stop=False)
            nc.tensor.matmul(pf2[:], onesf[0:1, :],
                             fc2b[:], start=False, stop=True)
            f2 = fcp.tile([64, 256], F32R)
            nc.scalar.activation(f2[:], pf2[:], AF.Relu, bias=zt[0:64, :])

            f2T = fcp.tile([128, 2*64], F32R)
            for k in range(2):
                ptr2 = psumF.tile([128, 64], F32R, tag="ptr", bufs=2)
                nc.tensor.transpose(ptr2[:], f2[:, k*128:(k+1)*128], identT[0:64, 0:64])
                nc.scalar.activation(f2T[:, k*64:(k+1)*64], ptr2[:], AF.Identity, bias=zt[:, :])

            fc3a = fcp.tile([128, 2*128], F32R)
            nc.sync.dma_start(fc3a[:], ap(fc3t, 0, [[128, 128], [128*128, 2], [1, 128]]))
            fc3b = fcp.tile([1, 128], F32R)
            nc.sync.dma_start(fc3b[:], fc3t[256:257, :])
            pf3 = psumF.tile([64, 128], F32, tag="pf3")
            for k in range(2):
                nc.tensor.matmul(pf3[:], f2T[:, k*64:(k+1)*64],
                                 fc3a[:, k*128:(k+1)*128],
                                 start=(k == 0), stop=False)
            nc.tensor.matmul(pf3[:], onesf[0:1, :],
                             fc3b[:], start=False, stop=True)
            fout = fcp.tile([64, 128], F32)
            nc.scalar.activation(fout[:], pf3[:], AF.Identity, bias=zt[0:64, :])
            nc.sync.dma_start(out[:], fout[:])

    nc.compile()
    return nc


# ---------------------------------------------------------------------------
# host side
# ---------------------------------------------------------------------------
def _bf16(x):
    import ml_dtypes
    return np.asarray(x, np.float32).astype(ml_dtypes.bfloat16)


def prep_inputs(x, conv1_w, conv1_b, bn_g, bn_b, pconv_w, pconv_b, W_caps,
                fc1_w, fc1_b, fc2_w, fc2_b, fc3_w, fc3_b):
    x = np.asarray(x, np.float32)
    s_bn = (np.asarray(bn_g) / np.sqrt(1.0 + 1e-5)).astype(np.float32)
    w1f = (np.asarray(conv1_w) * s_bn[:, None, None, None]).astype(np.float32)
    b1v = (np.asarray(conv1_b)*s_bn + np.asarray(bn_b)).astype(np.float32)
    w1m = np.ascontiguousarray(np.transpose(w1f, (1, 2, 3, 0)).reshape(243, 256))
    w2t = np.transpose(np.asarray(pconv_w, np.float32), (1, 2, 3, 0))  # [ic,ky,kx,oc]
    w2t = w2t.reshape(2, 128, 9, 9, 2, 128)
    w2m = _bf16(np.ascontiguousarray(
        np.transpose(w2t, (1, 0, 2, 3, 4, 5)).reshape(128, 2*81*2*128)))
    W_caps = np.asarray(W_caps, np.float32)
    wks = []
    for k in range(N_CORES):
        Wk = W_caps[RPC*k:RPC*(k+1)]            # [576, 10, 32, 16]
        # r = q4*144 + g ; partition = q4*32 + c ; col = g*160 + n*16 + o
        wkm = np.transpose(Wk.reshape(4, NG, ND, PC, DC), (0, 3, 1, 2, 4))
        wks.append(_bf16(np.ascontiguousarray(wkm.reshape(128, NG*ND*DC))))
    fc1m = np.concatenate([np.asarray(fc1_w).T, np.asarray(fc1_b)[None, :]], 0).astype(np.float32)
    fc2m = np.concatenate([np.asarray(fc2_w).T, np.asarray(fc2_b)[None, :]], 0).astype(np.float32)
    fc3m = np.concatenate([np.asarray(fc3_w).T, np.asarray(fc3_b)[None, :]], 0).astype(np.float32)
    identm = np.eye(128, dtype=np.float32)
    in_maps = []
    for k in range(N_CORES):
        in_maps.append({
            "xs": np.ascontiguousarray(x[BPC*k:BPC*(k+1)]),
            "w1": w1m, "b1": b1v, "w2": w2m,
            "b2": np.asarray(pconv_b, np.float32),
            "wk": wks[k],
            "fc1t": fc1m, "fc2t": fc2m, "fc3t": fc3m,
            "ident": identm, "onesd": np.ones((128, 64), np.float32),
        })
    return in_maps


_NC_CACHE = {}


def kernel(**inputs):
    if 'main' not in _NC_CACHE:
        # debug=True keeps the intermediate dump DMAs; they act as pipeline
        # spacers that enforce the A2A/transpose/DRAM-stage orderings the
        # scheduler otherwise misses (NaN without them), at ~0.1% HW cost.
        _NC_CACHE['main'] = build(debug=(os.environ.get('KDBG','1')=='1'),
                                  declare_dbg=(os.environ.get('KDECL','0')=='1') or None)
    nc = _NC_CACHE['main']
    in_maps = prep_inputs(**{k: np.asarray(v) for k, v in inputs.items()})
    res = run_bass_kernel_spmd(nc, in_maps, list(range(N_CORES)))
    return np.asarray(res.results[0]["out"], dtype=np.float32)


# revision 34
# speedup vs baseline: 1.0122x; 1.0122x over previous
"""CapsNet feature extractor on 8 Trainium2 NeuronCores (Bass/Tile).

Sharding: phase A (conv1 -> bn/relu -> pconv -> squash) is data-parallel over
batch (8 images/core). An AllToAll reshards u from batch-sharded to
routes-sharded (576 routes/core = 32 pconv output channels/core, since
route r = oc*18 + q under the raw row-major reshape). Phase B (u_hat, dynamic
routing) is routes-sharded with an AllReduce on s each iteration; the final
FC stack is computed redundantly on every core; core 0's output is returned.

v2: h is stored phase-split (4 polyphase sub-images) so the stride-2 pconv
reads contiguous rows; the u reshard uses a contiguous A2A payload + hardware
DMA-transpose into the (c, route-group) layout; u_hat is staged r-major
through DRAM with batched 320B-run DMAs instead of per-route scatters.
"""
import os
import sys
sys.path.insert(0, '/opt/trn_rl_repo')
import numpy as np
from contextlib import ExitStack

import concourse.bass as bass
import concourse.bacc as bacc
import concourse.mybir as mybir
from concourse import tile
from concourse.bass_utils import run_bass_kernel_spmd

dt = mybir.dt
AF = mybir.ActivationFunctionType
ALU = mybir.AluOpType
AX = mybir.AxisListType

N_CORES = 8
B = 64; IN_C = 3; IMG = 64
NCAP = 8; PC = 32; ND = 10; DC = 16; FEAT = 128
C1 = 56
PR = 24
ROUTES = 4608
BPC = B // N_CORES
RPC = ROUTES // N_CORES          # 576
NO = ND * DC                     # 160
NG = RPC // 4                    # 144 groups of 4 routes (one per q4 block)
NT = 5                           # u_hat partition tiles (4 x 128 + 1 x 64)
F32, BF16, F32R = dt.float32, dt.bfloat16, dt.float32r


def ap(t, offset, dims):
    """Manual access pattern; dims = [[step, count], ...] outer->inner, elems."""
    if isinstance(t, bass.AP):
        return bass.AP(tensor=t.tensor, offset=t.offset + offset,
                       ap=[list(d) for d in dims])
    return bass.AP(tensor=t, offset=offset, ap=[list(d) for d in dims])


def fap(tl, p0, pcnt, off, dims, pstep_mult=1):
    """AP into an SBUF tile AP `tl`: partition run [p0, p0+pcnt) with partition
    step `pstep_mult` rows, then free dims (offset `off` within partition)."""
    pstep = tl.ap[0][0]
    return bass.AP(tensor=tl.tensor, offset=tl.offset + p0*pstep + off,
                   ap=[[pstep*pstep_mult, pcnt]] + [list(d) for d in dims])


def _split9(pbase):
    if pbase + 9 <= 128:
        return [(0, pbase, 0, 9)]
    if pbase >= 128:
        return [(1, pbase - 128, 0, 9)]
    c = 128 - pbase
    return [(0, pbase, 0, c), (1, 0, c, 9 - c)]


def build(debug=False, declare_dbg=None):
    if declare_dbg is None:
        declare_dbg = debug
    nc = bacc.Bacc("TRN2", target_bir_lowering=False, debug=False,
                   num_devices=N_CORES)

    xs = nc.declare_dram_parameter("xs", [BPC, IN_C, IMG, IMG], F32R, isOutput=False)
    w1 = nc.declare_dram_parameter("w1", [243, 256], F32R, isOutput=False)
    b1 = nc.declare_dram_parameter("b1", [256], F32, isOutput=False)
    w2 = nc.declare_dram_parameter("w2", [128, 2*81*2*128], BF16, isOutput=False)
    b2 = nc.declare_dram_parameter("b2", [256], F32, isOutput=False)
    wk = nc.declare_dram_parameter("wk", [128, NG*NO], BF16, isOutput=False)
    fc1t = nc.declare_dram_parameter("fc1t", [161, 512], F32R, isOutput=False)
    fc2t = nc.declare_dram_parameter("fc2t", [513, 256], F32R, isOutput=False)
    fc3t = nc.declare_dram_parameter("fc3t", [257, 128], F32R, isOutput=False)
    ident = nc.declare_dram_parameter("ident", [128, 128], F32R, isOutput=False)
    onesd = nc.declare_dram_parameter("onesd", [128, 64], F32R, isOutput=False)
    out = nc.declare_dram_parameter("out", [B, FEAT], F32, isOutput=True)
    dbg = {}
    if declare_dbg:
        dbg['u_dbg'] = nc.declare_dram_parameter("u_dbg", [BPC, ROUTES, PC], BF16, isOutput=True)
        dbg['ua_dbg'] = nc.declare_dram_parameter("ua_dbg", [128, NG*B], BF16, isOutput=True)
        dbg['uh_dbg'] = nc.declare_dram_parameter("uh_dbg", [NT, 128, B*NO], BF16, isOutput=True)

    # A2A payload per dst core: [img(8), chb(8), q(18), q4(4), c(32)] bf16
    u_send = nc.dram_tensor("u_send", [N_CORES, BPC*RPC*PC], BF16)
    u_recv = nc.dram_tensor("u_recv", [N_CORES, BPC*RPC*PC], BF16)
    # u_hat staged r-major: [r(576), b(64), no(160)] bf16
    uh_dram = nc.dram_tensor("uh_dram", [RPC, B*NO], BF16)
    s_send = nc.dram_tensor("s_send", [3, B*NO], F32)
    s_recv = nc.dram_tensor("s_recv", [3, B*NO], F32)
    v_stage = nc.dram_tensor("v_stage", [ND, B*DC], F32R)
    groups = [list(range(N_CORES))]

    with tile.TileContext(nc) as tc, ExitStack() as top:
        consts = top.enter_context(tc.tile_pool(name="consts", bufs=1))

        b1t = consts.tile([128, 2], F32)
        nc.sync.dma_start(b1t[:], ap(b1, 0, [[1, 128], [128, 2]]))
        b2t = consts.tile([128, 2], F32)
        nc.sync.dma_start(b2t[:], ap(b2, 0, [[1, 128], [128, 2]]))
        onesb = consts.tile([128, 1], BF16)
        nc.vector.memset(onesb[:], 1.0)
        onesrow = consts.tile([1, 128], BF16)
        nc.vector.memset(onesrow[:], 1.0)
        onesf = consts.tile([128, 64], F32R)
        nc.sync.dma_start(onesf[:], onesd[:])
        identT = consts.tile([128, 128], F32R)
        nc.sync.dma_start(identT[:], ident[:])
        zt = consts.tile([128, 1], F32)
        nc.vector.memset(zt[:], 0.0)
        epst = consts.tile([128, 1], F32)
        nc.vector.memset(epst[:], 1e-8)

        # ============ PHASE A ============
        with tc.tile_pool(name="pha", bufs=1) as pha, \
             tc.tile_pool(name="img", bufs=2) as ipool, \
             tc.tile_pool(name="psumA", bufs=2, space="PSUM") as psumA:
            w1a = pha.tile([128, 256], F32R)
            nc.sync.dma_start(w1a[:], w1[0:128, :])
            w1b = pha.tile([128, 256], F32R)
            nc.sync.dma_start(w1b[0:115, :], w1[128:243, :])
            w2sb = pha.tile([128, 2*81*2*128], BF16)
            nc.sync.dma_start(w2sb[:], w2[:])

            for img in range(BPC):
                xa = ipool.tile([128, C1*C1], F32R, tag="xa")
                xb = ipool.tile([128, C1*C1], F32R, tag="xb")
                for ic in range(IN_C):
                    for ky in range(9):
                        pbase = ic*81 + ky*9
                        soff = img*IN_C*IMG*IMG + ic*IMG*IMG + ky*IMG
                        for (tdst, p0, kx0, cnt) in _split9(pbase):
                            dstt = xa if tdst == 0 else xb
                            nc.sync.dma_start(
                                dstt[p0:p0+cnt, :],
                                ap(xs, soff + kx0, [[1, cnt], [IMG, C1], [1, C1]]))

                # h phase-split: [128, ichg(2) x phase(4) x 784] bf16
                # phase = (y%2)*2 + (x%2); sub-image pos (y//2, x//2) in 28x28
                h = ipool.tile([128, 2*C1*C1], BF16, tag="h")
                for oyc in range(7):
                    for och in range(2):
                        ps = psumA.tile([128, 448], F32, tag="psc1")
                        nc.tensor.matmul(ps[:], w1a[:, och*128:(och+1)*128],
                                         xa[:, oyc*448:(oyc+1)*448],
                                         start=True, stop=False)
                        nc.tensor.matmul(ps[:], w1b[0:115, och*128:(och+1)*128],
                                         xb[0:115, oyc*448:(oyc+1)*448],
                                         start=False, stop=True)
                        for py in range(2):
                            for px in range(2):
                                nc.scalar.activation(
                                    fap(h, 0, 128,
                                        och*3136 + (py*2+px)*784 + oyc*112,
                                        [[28, 4], [1, 28]]),
                                    fap(ps, 0, 128, py*56 + px,
                                        [[112, 4], [2, 28]]),
                                    AF.Relu, bias=b1t[:, och:och+1])

                # pconv: accumulate 81 taps from the 4 phase sub-images
                pst = ipool.tile([128, 2*PR*PR], F32, tag="pst")
                for och in range(2):
                    psA = psumA.tile([128, 288], F32, tag="psA")
                    psB = psumA.tile([128, 288], F32, tag="psB")
                    for ich in range(2):
                        for ky in range(9):
                            for kx in range(9):
                                widx = ((ich*81 + ky*9 + kx)*2 + och)*128
                                lhsT = w2sb[:, widx:widx+128]
                                first = (ich == 0 and ky == 0 and kx == 0)
                                last = (ich == 1 and ky == 8 and kx == 8)
                                base = (ich*3136 + ((ky % 2)*2 + (kx % 2))*784
                                        + (ky//2)*28 + (kx//2))
                                nc.tensor.matmul(
                                    psA[:], lhsT,
                                    fap(h, 0, 128, base, [[28, 12], [1, 24]]),
                                    start=first, stop=last)
                                nc.tensor.matmul(
                                    psB[:], lhsT,
                                    fap(h, 0, 128, base + 12*28, [[28, 12], [1, 24]]),
                                    start=first, stop=last)
                    nc.scalar.activation(pst[:, och*576:och*576+288], psA[:],
                                         AF.Identity, bias=b2t[:, och:och+1])
                    nc.scalar.activation(pst[:, och*576+288:och*576+576], psB[:],
                                         AF.Identity, bias=b2t[:, och:och+1])

                p2 = ipool.tile([128, 2*PR*PR], F32, tag="p2")
                nc.vector.tensor_mul(p2[:], pst[:], pst[:])
                sn = ipool.tile([128, 36], F32, tag="sn")
                nc.vector.tensor_reduce(sn[:], fap(p2, 0, 128, 0, [[32, 36], [1, 32]]),
                                        AX.X, ALU.add)
                sn1 = ipool.tile([128, 36], F32, tag="sn1")
                nc.vector.tensor_scalar_add(sn1[:], sn[:], 1.0)
                rde = ipool.tile([128, 36], F32, tag="rde")
                nc.vector.reciprocal(rde[:], sn1[:])
                sqr = ipool.tile([128, 36], F32, tag="sqr")
                nc.scalar.activation(sqr[:], sn[:], AF.Sqrt, bias=epst[:, :])
                rsq = ipool.tile([128, 36], F32, tag="rsq")
                nc.vector.reciprocal(rsq[:], sqr[:])
                scl = ipool.tile([128, 36], F32, tag="scl")
                nc.vector.tensor_mul(scl[:], sn[:], rde[:])
                nc.vector.tensor_mul(scl[:], scl[:], rsq[:])
                usq = ipool.tile([128, 2*PR*PR], BF16, tag="usq")
                nc.vector.tensor_mul(usq[:], pst[:],
                                     fap(scl, 0, 128, 0, [[1, 36], [0, 32]]))
                if debug:
                    for och in range(2):
                        nc.sync.dma_start(
                            ap(dbg['u_dbg'], img*ROUTES*PC + och*128*576,
                               [[576, 128], [1, 576]]),
                            usq[:, och*576:(och+1)*576])
                # send: dst core j = och*4+kq gets channels 32j..32j+32.
                # payload layout [img, chb(8), q(18), q4(4), c(32)]:
                #   addr = ((img*8+chb)*18+q)*128 + q4*32 + c,  ch_loc = 8*q4+chb
                for kq in range(4):
                    for och in range(2):
                        j = och*4 + kq
                        nc.sync.dma_start(
                            ap(u_send, j*BPC*RPC*PC + img*RPC*PC,
                               [[32, 4], [18*128, 8], [128, 18], [1, 32]]),
                            fap(usq, 32*kq, 32, och*576, [[32, 18], [1, 32]]))

        nc.gpsimd.collective_compute("AllToAll", ALU.bypass, replica_groups=groups,
                                     ins=[u_send[:]], outs=[u_recv[:]])

        # ============ PHASE B: u_hat ============
        uhp = top.enter_context(tc.tile_pool(name="uhp", bufs=1))
        uh = [uhp.tile([128, B*NO], BF16, name=f"uh{t}", tag=f"uh{t}")
              for t in range(NT)]
        nc.vector.memset(uh[4][64:128, :], 0.0)

        with tc.tile_pool(name="utp", bufs=1) as utp, \
             tc.tile_pool(name="wstr", bufs=2) as wstr, \
             tc.tile_pool(name="stgp", bufs=2) as stgp, \
             tc.tile_pool(name="psumB", bufs=4, space="PSUM") as psumB:
            # u_all128[p=(q4,c), col=b*144+g] via HW dma-transpose per src:
            # in [1152 rows=(img,chb,q), 128 cols=(q4,c)] -> out [128, 1152]
            u_all = utp.tile([128, NG*B], BF16)
            for src in range(N_CORES):
                nc.sync.dma_start(
                    u_all[:, src*1152:(src+1)*1152],
                    ap(u_recv, src*BPC*RPC*PC, [[128, 1152], [1, 128]]),
                    transpose=True)

            if debug:
                nc.sync.dma_start(dbg['ua_dbg'][:], u_all[:])
            tc.strict_bb_all_engine_barrier()
            # r = q4*144 + g ; lhsT = u_all[32q4:32q4+32, cols b*144+g] (64 b)
            # rhs = wchunk[32q4:32q4+32, (g%24)*160 : +160]
            GCH = 24                       # g-groups per W chunk
            for ck in range(NG // GCH):    # 6 W chunks, each reused by 4 q4
                wchunk = wstr.tile([128, GCH*NO], BF16, tag="wchunk")
                nc.sync.dma_start(wchunk[:], wk[:, ck*GCH*NO:(ck+1)*GCH*NO])
                for q4 in range(4):
                    # 24 consecutive routes rb..rb+23 (r-major within block)
                    rb = q4*NG + ck*GCH
                    stgt = stgp.tile([64, GCH*NO], BF16, tag="stgt")
                    for i3 in range(GCH // 3):
                        psr = psumB.tile([64, 3*NO], F32, tag="psr")
                        for j in range(3):
                            i = i3*3 + j
                            g = ck*GCH + i
                            nc.tensor.matmul(
                                psr[:, j*NO:(j+1)*NO],
                                fap(u_all, 32*q4, 32, g, [[NG, B]]),
                                wchunk[32*q4:32*q4+32, i*NO:(i+1)*NO],
                                start=True, stop=True, tile_position=(32*q4, 0))
                        nc.scalar.activation(stgt[:, i3*3*NO:(i3+1)*3*NO],
                                             psr[:], AF.Identity, bias=zt[0:64, :])
                    nc.sync.dma_start(
                        ap(uh_dram, rb*B*NO,
                           [[NO, B], [B*NO, GCH], [1, NO]]),
                        stgt[:])
            tc.strict_bb_all_engine_barrier()
            for t in range(NT):
                rows = 128 if t < 4 else 64
                nc.sync.dma_start(
                    uh[t][0:rows, :],
                    ap(uh_dram, t*128*B*NO, [[B*NO, rows], [1, B*NO]]))
                if debug:
                    nc.sync.dma_start(
                        ap(dbg['uh_dbg'], t*128*B*NO, [[B*NO, rows], [1, B*NO]]),
                        uh[t][0:rows, :])

        # ============ routing ============
        rp = top.enter_context(tc.tile_pool(name="rp", bufs=1))
        logits = rp.tile([128, NT*B*ND], F32)
        nc.vector.memset(logits[:], 0.0)
        c_t = rp.tile([128, NT*B*ND], BF16)
        s_sb = rp.tile([10, B*DC], F32)
        v_sb = rp.tile([10, B*DC], F32)
        vb16 = rp.tile([10, B*DC], BF16)
        sq2 = rp.tile([10, B*DC], F32)
        snv = rp.tile([10, B], F32)
        snv1 = rp.tile([10, B], F32)
        rdev = rp.tile([10, B], F32)
        sqv = rp.tile([10, B], F32)
        rsqv = rp.tile([10, B], F32)
        sclv = rp.tile([10, B], F32)
        sume = rp.tile([128, NT*B], F32)
        rece = rp.tile([128, NT*B], F32)

        with tc.tile_pool(name="agg", bufs=1) as agg, \
             tc.tile_pool(name="m2cp", bufs=3) as m2cp, \
             tc.tile_pool(name="sstg", bufs=4) as sstg, \
             tc.tile_pool(name="psumR", bufs=4, space="PSUM") as psumR:
            m2b = agg.tile([128, B*NO//2], BF16)
            red = agg.tile([128, B*ND], F32)
            v_rep = agg.tile([128, B*NO], BF16)
            vrow = agg.tile([1, B*NO], BF16)

            for it in range(3):
                # ---- partial s = sum over local routes of c * u_hat ----
                for chk in range(22):
                    b0 = chk*3
                    nb = min(3, B - b0)
                    w = nb*NO
                    pss = psumR.tile([128, 512], F32, tag="pss")
                    for t in range(NT):
                        kk = 128 if t < 4 else 64
                        if it == 0:
                            rhs = uh[t][0:kk, b0*NO:b0*NO + w]
                        else:
                            m2c = m2cp.tile([128, 512], BF16, tag="m2c")
                            nc.vector.tensor_mul(
                                m2c[0:kk, 0:w], uh[t][0:kk, b0*NO:b0*NO + w],
                                fap(c_t, 0, kk, t*B*ND + b0*ND,
                                    [[ND, nb], [1, ND], [0, DC]]))
                            rhs = m2c[0:kk, 0:w]
                        nc.tensor.matmul(pss[0:1, 0:w], onesb[0:kk, :], rhs,
                                         start=(t == 0), stop=(t == 4))
                    sst = sstg.tile([1, 512], F32, tag="sst")
                    nc.scalar.activation(sst[0:1, 0:w], pss[0:1, 0:w],
                                         AF.Identity, bias=zt[0:1, :],
                                         scale=(0.1 if it == 0 else 1.0))
                    nc.sync.dma_start(ap(s_send, it*B*NO + b0*NO, [[1, w]]),
                                      sst[0:1, 0:w])
                nc.gpsimd.collective_compute(
                    "AllReduce", ALU.add, replica_groups=groups,
                    ins=[ap(s_send, it*B*NO, [[1, B*NO]])],
                    outs=[ap(s_recv, it*B*NO, [[1, B*NO]])])

                # ---- v = squash(s) in [10p=n, (b, o)] ----
                nc.sync.dma_start(s_sb[:],
                                  ap(s_recv, it*B*NO, [[DC, ND], [NO, B], [1, DC]]))
                nc.vector.tensor_mul(sq2[:], s_sb[:], s_sb[:])
                nc.vector.tensor_reduce(snv[:], fap(sq2, 0, 10, 0, [[DC, B], [1, DC]]),
                                        AX.X, ALU.add)
                nc.vector.tensor_scalar_add(snv1[:], snv[:], 1.0)
                nc.vector.reciprocal(rdev[:], snv1[:])
                nc.scalar.activation(sqv[:], snv[:], AF.Sqrt, bias=epst[0:10, :])
                nc.vector.reciprocal(rsqv[:], sqv[:])
                nc.vector.tensor_mul(sclv[:], snv[:], rdev[:])
                nc.vector.tensor_mul(sclv[:], sclv[:], rsqv[:])
                nc.vector.tensor_mul(v_sb[:], s_sb[:],
                                     fap(sclv, 0, 10, 0, [[1, B], [0, DC]]))
                if it == 2:
                    nc.sync.dma_start(v_stage[:].bitcast(F32), v_sb[:])
                    break
                nc.vector.tensor_copy(vb16[:], v_sb[:])
                # vrow [1, (b,n,o)] then broadcast to 128 partitions via PE
                for n in range(ND):
                    nc.sync.dma_start(
                        fap(vrow, 0, 1, n*DC, [[NO, B], [1, DC]]),
                        vb16[n:n+1, :])
                for k in range(B*NO//512):
                    psv = psumR.tile([128, 512], F32, tag="psv")
                    nc.tensor.matmul(psv[:], onesrow[0:1, :],
                                     vrow[0:1, k*512:(k+1)*512],
                                     start=True, stop=True)
                    nc.scalar.activation(v_rep[:, k*512:(k+1)*512], psv[:],
                                         AF.Identity, bias=zt[:, :])

                # ---- agreement: logits += sum_o u_hat * v (half-tiles) ----
                for t in range(NT):
                    for hh in range(2):
                        sl = slice(hh*B*NO//2, (hh+1)*B*NO//2)
                        nc.vector.tensor_mul(m2b[:], uh[t][:, sl], v_rep[:, sl])
                        nc.vector.tensor_reduce(
                            red[:, hh*B*ND//2:(hh+1)*B*ND//2],
                            fap(m2b, 0, 128, 0, [[DC, B*ND//2], [1, DC]]),
                            AX.X, ALU.add)
                    nc.vector.tensor_add(
                        logits[:, t*B*ND:(t+1)*B*ND],
                        logits[:, t*B*ND:(t+1)*B*ND],
                        red[:])

                # ---- c = softmax(logits) over n (no max-subtraction) ----
                eexp = v_rep[:].bitcast(F32)        # [128, 5120] f32 view
                nc.scalar.activation(eexp[:, 0:NT*B*ND], logits[:], AF.Exp, bias=zt[:, :])
                nc.vector.tensor_reduce(
                    sume[:], fap(eexp, 0, 128, 0, [[ND, NT*B], [1, ND]]),
                    AX.X, ALU.add)
                nc.vector.reciprocal(rece[:], sume[:])
                nc.vector.tensor_mul(c_t[:], eexp[:, 0:NT*B*ND],
                                     fap(rece, 0, 128, 0, [[1, NT*B], [0, ND]]))

        # ============ FC head (redundant on every core) ============
        with tc.tile_pool(name="fcp", bufs=1) as fcp, \
             tc.tile_pool(name="psumF", bufs=1, space="PSUM") as psumF:
            fta = fcp.tile([128, B], F32R)
            ftb = fcp.tile([128, B], F32R)
            for n in range(ND):
                dstt, p0 = (fta, n*DC) if n < 8 else (ftb, (n-8)*DC)
                nc.sync.dma_start(dstt[p0:p0+DC, :],
                                  ap(v_stage, n*B*DC, [[1, DC], [DC, B]]))
            nc.sync.dma_start(ftb[32:33, :], onesd[0:1, :])

            fc1a = fcp.tile([128, 512], F32R)
            nc.sync.dma_start(fc1a[:], fc1t[0:128, :])
            fc1b = fcp.tile([128, 512], F32R)
            nc.sync.dma_start(fc1b[0:33, :], fc1t[128:161, :])
            pf1 = psumF.tile([64, 512], F32, tag="pf1")
            nc.tensor.matmul(pf1[:], fta[:, 0:64],
                             fc1a[:], start=True, stop=False)
            nc.tensor.matmul(pf1[:], ftb[0:33, 0:64],
                             fc1b[0:33, :], start=False, stop=True)
            f1 = fcp.tile([64, 512], F32R)
            nc.scalar.activation(f1[:], pf1[:], AF.Relu, bias=zt[0:64, :])

            f1T = fcp.tile([128, 4*64], F32R)
            for k in range(4):
                ptr = psumF.tile([128, 64], F32R, tag="ptr", bufs=2)
                nc.tensor.transpose(ptr[:], f1[:, k*128:(k+1)*128], identT[0:64, 0:64])
                nc.scalar.activation(f1T[:, k*64:(k+1)*64], ptr[:], AF.Identity, bias=zt[:, :])

            fc2a = fcp.tile([128, 4*256], F32R)
            nc.sync.dma_start(fc2a[:], ap(fc2t, 0, [[256, 128], [128*256, 4], [1, 256]]))
            fc2b = fcp.tile([1, 256], F32R)
            nc.sync.dma_start(fc2b[:], fc2t[512:513, :])
            pf2 = psumF.tile([64, 256], F32, tag="pf2")
            for k in range(4):
                nc.tensor.matmul(pf2[:], f1T[:, k*64:(k+1)*64],
                                 fc2a[:, k*256:(k+1)*256],
                                 start=(k == 0), stop=False)
            nc.tensor.matmul(pf2[:], onesf[0:1, :],
                             fc2b[:], start=False, stop=True)
            f2 = fcp.tile([64, 256], F32R)
            nc.scalar.activation(f2[:], pf2[:], AF.Relu, bias=zt[0:64, :])

            f2T = fcp.tile([128, 2*64], F32R)
            for k in range(2):
                ptr2 = psumF.tile([128, 64], F32R, tag="ptr", bufs=2)
                nc.tensor.transpose(ptr2[:], f2[:, k*128:(k+1)*128], identT[0:64, 0:64])
                nc.scalar.activation(f2T[:, k*64:(k+1)*64], ptr2[:], AF.Identity, bias=zt[:, :])

            fc3a = fcp.tile([128, 2*128], F32R)
            nc.sync.dma_start(fc3a[:], ap(fc3t, 0, [[128, 128], [128*128, 2], [1, 128]]))
            fc3b = fcp.tile([1, 128], F32R)
            nc.sync.dma_start(fc3b[:], fc3t[256:257, :])
            pf3 = psumF.tile([64, 128], F32, tag="pf3")
            for k in range(2):
                nc.tensor.matmul(pf3[:], f2T[:, k*64:(k+1)*64],
                                 fc3a[:, k*128:(k+1)*128],
                                 start=(k == 0), stop=False)
            nc.tensor.matmul(pf3[:], onesf[0:1, :],
                             fc3b[:], start=False, stop=True)
            fout = fcp.tile([64, 128], F32)
            nc.scalar.activation(fout[:], pf3[:], AF.Identity, bias=zt[0:64, :])
            nc.sync.dma_start(out[:], fout[:])

    nc.compile()
    return nc


# ---------------------------------------------------------------------------
# host side
# ---------------------------------------------------------------------------
def _bf16(x):
    import ml_dtypes
    return np.asarray(x, np.float32).astype(ml_dtypes.bfloat16)


def prep_inputs(x, conv1_w, conv1_b, bn_g, bn_b, pconv_w, pconv_b, W_caps,
                fc1_w, fc1_b, fc2_w, fc2_b, fc3_w, fc3_b):
    x = np.asarray(x, np.float32)
    s_bn = (np.asarray(bn_g) / np.sqrt(1.0 + 1e-5)).astype(np.float32)
    w1f = (np.asarray(conv1_w) * s_bn[:, None, None, None]).astype(np.float32)
    b1v = (np.asarray(conv1_b)*s_bn + np.asarray(bn_b)).astype(np.float32)
    w1m = np.ascontiguousarray(np.transpose(w1f, (1, 2, 3, 0)).reshape(243, 256))
    w2t = np.transpose(np.asarray(pconv_w, np.float32), (1, 2, 3, 0))  # [ic,ky,kx,oc]
    w2t = w2t.reshape(2, 128, 9, 9, 2, 128)
    w2m = _bf16(np.ascontiguousarray(
        np.transpose(w2t, (1, 0, 2, 3, 4, 5)).reshape(128, 2*81*2*128)))
    W_caps = np.asarray(W_caps, np.float32)
    wks = []
    for k in range(N_CORES):
        Wk = W_caps[RPC*k:RPC*(k+1)]            # [576, 10, 32, 16]
        # r = q4*144 + g ; partition = q4*32 + c ; col = g*160 + n*16 + o
        wkm = np.transpose(Wk.reshape(4, NG, ND, PC, DC), (0, 3, 1, 2, 4))
        wks.append(_bf16(np.ascontiguousarray(wkm.reshape(128, NG*ND*DC))))
    fc1m = np.concatenate([np.asarray(fc1_w).T, np.asarray(fc1_b)[None, :]], 0).astype(np.float32)
    fc2m = np.concatenate([np.asarray(fc2_w).T, np.asarray(fc2_b)[None, :]], 0).astype(np.float32)
    fc3m = np.concatenate([np.asarray(fc3_w).T, np.asarray(fc3_b)[None, :]], 0).astype(np.float32)
    identm = np.eye(128, dtype=np.float32)
    in_maps = []
    for k in range(N_CORES):
        in_maps.append({
            "xs": np.ascontiguousarray(x[BPC*k:BPC*(k+1)]),
            "w1": w1m, "b1": b1v, "w2": w2m,
            "b2": np.asarray(pconv_b, np.float32),
            "wk": wks[k],
            "fc1t": fc1m, "fc2t": fc2m, "fc3t": fc3m,
            "ident": identm, "onesd": np.ones((128, 64), np.float32),
        })
    return in_maps


_NC_CACHE = {}


def kernel(**inputs):
    if 'main' not in _NC_CACHE:
        # debug=True keeps the intermediate dump DMAs; they act as pipeline
        # spacers that enforce the A2A/transpose/DRAM-stage orderings the
        # scheduler otherwise misses (NaN without them), at ~0.1% HW cost.
        _NC_CACHE['main'] = build(debug=(os.environ.get('KDBG','1')=='1'),
                                  declare_dbg=(os.environ.get('KDECL','0')=='1') or None)
    nc = _NC_CACHE['main']
    in_maps = prep_inputs(**{k: np.asarray(v) for k, v in inputs.items()})
    res = run_bass_kernel_spmd(nc, in_maps, list(range(N_CORES)))
    return np.asarray(res.results[0]["out"], dtype=np.float32)


# revision 38
# speedup vs baseline: 1.0125x; 1.0003x over previous
"""CapsNet feature extractor on 8 Trainium2 NeuronCores (Bass/Tile).

Sharding: phase A (conv1 -> bn/relu -> pconv -> squash) is data-parallel over
batch (8 images/core). An AllToAll reshards u from batch-sharded to
routes-sharded (576 routes/core = 32 pconv output channels/core, since
route r = oc*18 + q under the raw row-major reshape). Phase B (u_hat, dynamic
routing) is routes-sharded with an AllReduce on s each iteration; the final
FC stack is computed redundantly on every core; core 0's output is returned.

v2: h is stored phase-split (4 polyphase sub-images) so the stride-2 pconv
reads contiguous rows; the u reshard uses a contiguous A2A payload + hardware
DMA-transpose into the (c, route-group) layout; u_hat is staged r-major
through DRAM with batched 320B-run DMAs instead of per-route scatters.
"""
import os
import sys
sys.path.insert(0, '/opt/trn_rl_repo')
import numpy as np
from contextlib import ExitStack

import concourse.bass as bass
import concourse.bacc as bacc
import concourse.mybir as mybir
from concourse import tile
from concourse.bass_utils import run_bass_kernel_spmd

dt = mybir.dt
AF = mybir.ActivationFunctionType
ALU = mybir.AluOpType
AX = mybir.AxisListType

N_CORES = 8
B = 64; IN_C = 3; IMG = 64
NCAP = 8; PC = 32; ND = 10; DC = 16; FEAT = 128
C1 = 56
PR = 24
ROUTES = 4608
BPC = B // N_CORES
RPC = ROUTES // N_CORES          # 576
NO = ND * DC                     # 160
NG = RPC // 4                    # 144 groups of 4 routes (one per q4 block)
NT = 5                           # u_hat partition tiles (4 x 128 + 1 x 64)
F32, BF16, F32R = dt.float32, dt.bfloat16, dt.float32r


def ap(t, offset, dims):
    """Manual access pattern; dims = [[step, count], ...] outer->inner, elems."""
    if isinstance(t, bass.AP):
        return bass.AP(tensor=t.tensor, offset=t.offset + offset,
                       ap=[list(d) for d in dims])
    return bass.AP(tensor=t, offset=offset, ap=[list(d) for d in dims])


def fap(tl, p0, pcnt, off, dims, pstep_mult=1):
    """AP into an SBUF tile AP `tl`: partition run [p0, p0+pcnt) with partition
    step `pstep_mult` rows, then free dims (offset `off` within partition)."""
    pstep = tl.ap[0][0]
    return bass.AP(tensor=tl.tensor, offset=tl.offset + p0*pstep + off,
                   ap=[[pstep*pstep_mult, pcnt]] + [list(d) for d in dims])


def _split9(pbase):
    if pbase + 9 <= 128:
        return [(0, pbase, 0, 9)]
    if pbase >= 128:
        return [(1, pbase - 128, 0, 9)]
    c = 128 - pbase
    return [(0, pbase, 0, c), (1, 0, c, 9 - c)]


def build(debug=False, declare_dbg=None):
    if declare_dbg is None:
        declare_dbg = debug
    nc = bacc.Bacc("TRN2", target_bir_lowering=False, debug=False,
                   num_devices=N_CORES)

    xs = nc.declare_dram_parameter("xs", [BPC, IN_C, IMG, IMG], F32R, isOutput=False)
    w1 = nc.declare_dram_parameter("w1", [243, 256], F32R, isOutput=False)
    b1 = nc.declare_dram_parameter("b1", [256], F32, isOutput=False)
    w2 = nc.declare_dram_parameter("w2", [128, 2*81*2*128], BF16, isOutput=False)
    b2 = nc.declare_dram_parameter("b2", [256], F32, isOutput=False)
    wk = nc.declare_dram_parameter("wk", [128, NG*NO], BF16, isOutput=False)
    fc1t = nc.declare_dram_parameter("fc1t", [161, 512], F32R, isOutput=False)
    fc2t = nc.declare_dram_parameter("fc2t", [513, 256], F32R, isOutput=False)
    fc3t = nc.declare_dram_parameter("fc3t", [257, 128], F32R, isOutput=False)
    ident = nc.declare_dram_parameter("ident", [128, 128], F32R, isOutput=False)
    onesd = nc.declare_dram_parameter("onesd", [128, 64], F32R, isOutput=False)
    out = nc.declare_dram_parameter("out", [B, FEAT], F32, isOutput=True)
    dbg = {}
    if declare_dbg:
        dbg['u_dbg'] = nc.declare_dram_parameter("u_dbg", [BPC, ROUTES, PC], BF16, isOutput=True)
        dbg['ua_dbg'] = nc.declare_dram_parameter("ua_dbg", [128, NG*B], BF16, isOutput=True)
        dbg['uh_dbg'] = nc.declare_dram_parameter("uh_dbg", [NT, 128, B*NO], BF16, isOutput=True)

    # A2A payload per dst core: [img(8), chb(8), q(18), q4(4), c(32)] bf16
    u_send = nc.dram_tensor("u_send", [N_CORES, BPC*RPC*PC], BF16)
    u_recv = nc.dram_tensor("u_recv", [N_CORES, BPC*RPC*PC], BF16)
    # u_hat staged r-major: [r(576), b(64), no(160)] bf16
    uh_dram = nc.dram_tensor("uh_dram", [RPC, B*NO], BF16)
    s_send = nc.dram_tensor("s_send", [3, B*NO], F32)
    s_recv = nc.dram_tensor("s_recv", [3, B*NO], F32)
    v_stage = nc.dram_tensor("v_stage", [ND, B*DC], F32R)
    groups = [list(range(N_CORES))]

    with tile.TileContext(nc) as tc, ExitStack() as top:
        consts = top.enter_context(tc.tile_pool(name="consts", bufs=1))

        b1t = consts.tile([128, 2], F32)
        nc.sync.dma_start(b1t[:], ap(b1, 0, [[1, 128], [128, 2]]))
        b2t = consts.tile([128, 2], F32)
        nc.sync.dma_start(b2t[:], ap(b2, 0, [[1, 128], [128, 2]]))
        onesb = consts.tile([128, 1], BF16)
        nc.vector.memset(onesb[:], 1.0)
        onesrow = consts.tile([1, 128], BF16)
        nc.vector.memset(onesrow[:], 1.0)
        onesf = consts.tile([128, 64], F32R)
        nc.sync.dma_start(onesf[:], onesd[:])
        identT = consts.tile([128, 128], F32R)
        nc.sync.dma_start(identT[:], ident[:])
        zt = consts.tile([128, 1], F32)
        nc.vector.memset(zt[:], 0.0)
        epst = consts.tile([128, 1], F32)
        nc.vector.memset(epst[:], 1e-8)

        # ============ PHASE A ============
        with tc.tile_pool(name="pha", bufs=1) as pha, \
             tc.tile_pool(name="img", bufs=2) as ipool, \
             tc.tile_pool(name="psumA", bufs=2, space="PSUM") as psumA:
            w1a = pha.tile([128, 256], F32R)
            nc.sync.dma_start(w1a[:], w1[0:128, :])
            w1b = pha.tile([128, 256], F32R)
            nc.sync.dma_start(w1b[0:115, :], w1[128:243, :])
            w2sb = pha.tile([128, 2*81*2*128], BF16)
            nc.sync.dma_start(w2sb[:], w2[:])

            for img in range(BPC):
                xa = ipool.tile([128, C1*C1], F32R, tag="xa")
                xb = ipool.tile([128, C1*C1], F32R, tag="xb")
                for ic in range(IN_C):
                    for ky in range(9):
                        pbase = ic*81 + ky*9
                        soff = img*IN_C*IMG*IMG + ic*IMG*IMG + ky*IMG
                        for (tdst, p0, kx0, cnt) in _split9(pbase):
                            dstt = xa if tdst == 0 else xb
                            nc.sync.dma_start(
                                dstt[p0:p0+cnt, :],
                                ap(xs, soff + kx0, [[1, cnt], [IMG, C1], [1, C1]]))

                # h phase-split: [128, ichg(2) x phase(4) x 784] bf16
                # phase = (y%2)*2 + (x%2); sub-image pos (y//2, x//2) in 28x28
                h = ipool.tile([128, 2*C1*C1], BF16, tag="h")
                for oyc in range(7):
                    for och in range(2):
                        ps = psumA.tile([128, 448], F32, tag="psc1")
                        nc.tensor.matmul(ps[:], w1a[:, och*128:(och+1)*128],
                                         xa[:, oyc*448:(oyc+1)*448],
                                         start=True, stop=False)
                        nc.tensor.matmul(ps[:], w1b[0:115, och*128:(och+1)*128],
                                         xb[0:115, oyc*448:(oyc+1)*448],
                                         start=False, stop=True)
                        for py in range(2):
                            for px in range(2):
                                nc.scalar.activation(
                                    fap(h, 0, 128,
                                        och*3136 + (py*2+px)*784 + oyc*112,
                                        [[28, 4], [1, 28]]),
                                    fap(ps, 0, 128, py*56 + px,
                                        [[112, 4], [2, 28]]),
                                    AF.Relu, bias=b1t[:, och:och+1])

                # pconv: accumulate 81 taps from the 4 phase sub-images
                pst = ipool.tile([128, 2*PR*PR], F32, tag="pst")
                for och in range(2):
                    psA = psumA.tile([128, 288], F32, tag="psA")
                    psB = psumA.tile([128, 288], F32, tag="psB")
                    for ich in range(2):
                        for ky in range(9):
                            for kx in range(9):
                                widx = ((ich*81 + ky*9 + kx)*2 + och)*128
                                lhsT = w2sb[:, widx:widx+128]
                                first = (ich == 0 and ky == 0 and kx == 0)
                                last = (ich == 1 and ky == 8 and kx == 8)
                                base = (ich*3136 + ((ky % 2)*2 + (kx % 2))*784
                                        + (ky//2)*28 + (kx//2))
                                nc.tensor.matmul(
                                    psA[:], lhsT,
                                    fap(h, 0, 128, base, [[28, 12], [1, 24]]),
                                    start=first, stop=last)
                                nc.tensor.matmul(
                                    psB[:], lhsT,
                                    fap(h, 0, 128, base + 12*28, [[28, 12], [1, 24]]),
                                    start=first, stop=last)
                    nc.scalar.activation(pst[:, och*576:och*576+288], psA[:],
                                         AF.Identity, bias=b2t[:, och:och+1])
                    nc.scalar.activation(pst[:, och*576+288:och*576+576], psB[:],
                                         AF.Identity, bias=b2t[:, och:och+1])

                p2 = ipool.tile([128, 2*PR*PR], F32, tag="p2")
                nc.vector.tensor_mul(p2[:], pst[:], pst[:])
                sn = ipool.tile([128, 36], F32, tag="sn")
                nc.vector.tensor_reduce(sn[:], fap(p2, 0, 128, 0, [[32, 36], [1, 32]]),
                                        AX.X, ALU.add)
                sn1 = ipool.tile([128, 36], F32, tag="sn1")
                nc.vector.tensor_scalar_add(sn1[:], sn[:], 1.0)
                rde = ipool.tile([128, 36], F32, tag="rde")
                nc.vector.reciprocal(rde[:], sn1[:])
                sqr = ipool.tile([128, 36], F32, tag="sqr")
                nc.scalar.activation(sqr[:], sn[:], AF.Sqrt, bias=epst[:, :])
                rsq = ipool.tile([128, 36], F32, tag="rsq")
                nc.vector.reciprocal(rsq[:], sqr[:])
                scl = ipool.tile([128, 36], F32, tag="scl")
                nc.vector.tensor_mul(scl[:], sn[:], rde[:])
                nc.vector.tensor_mul(scl[:], scl[:], rsq[:])
                usq = ipool.tile([128, 2*PR*PR], BF16, tag="usq")
                nc.vector.tensor_mul(usq[:], pst[:],
                                     fap(scl, 0, 128, 0, [[1, 36], [0, 32]]))
                if debug:
                    for och in range(2):
                        nc.sync.dma_start(
                            ap(dbg['u_dbg'], img*ROUTES*PC + och*128*576,
                               [[576, 128], [1, 576]]),
                            usq[:, och*576:(och+1)*576])
                # send: dst core j = och*4+kq gets channels 32j..32j+32.
                # payload layout [img, chb(8), q(18), q4(4), c(32)]:
                #   addr = ((img*8+chb)*18+q)*128 + q4*32 + c,  ch_loc = 8*q4+chb
                for kq in range(4):
                    for och in range(2):
                        j = och*4 + kq
                        nc.sync.dma_start(
                            ap(u_send, j*BPC*RPC*PC + img*RPC*PC,
                               [[32, 4], [18*128, 8], [128, 18], [1, 32]]),
                            fap(usq, 32*kq, 32, och*576, [[32, 18], [1, 32]]))

        nc.gpsimd.collective_compute("AllToAll", ALU.bypass, replica_groups=groups,
                                     ins=[u_send[:]], outs=[u_recv[:]])

        # ============ PHASE B: u_hat ============
        uhp = top.enter_context(tc.tile_pool(name="uhp", bufs=1))
        uh = [uhp.tile([128, B*NO], BF16, name=f"uh{t}", tag=f"uh{t}")
              for t in range(NT)]
        nc.vector.memset(uh[4][64:128, :], 0.0)

        with tc.tile_pool(name="utp", bufs=1) as utp, \
             tc.tile_pool(name="wstr", bufs=2) as wstr, \
             tc.tile_pool(name="stgp", bufs=2) as stgp, \
             tc.tile_pool(name="psumB", bufs=4, space="PSUM") as psumB:
            # u_all128[p=(q4,c), col=b*144+g] via HW dma-transpose per src:
            # in [1152 rows=(img,chb,q), 128 cols=(q4,c)] -> out [128, 1152]
            u_all = utp.tile([128, NG*B], BF16)
            for src in range(N_CORES):
                nc.sync.dma_start(
                    u_all[:, src*1152:(src+1)*1152],
                    ap(u_recv, src*BPC*RPC*PC, [[128, 1152], [1, 128]]),
                    transpose=True)

            if debug:
                nc.sync.dma_start(dbg['ua_dbg'][:], u_all[:])
            tc.strict_bb_all_engine_barrier()
            # r = q4*144 + g ; lhsT = u_all[32q4:32q4+32, cols b*144+g] (64 b)
            # rhs = wchunk[32q4:32q4+32, (g%24)*160 : +160]
            GCH = 24                       # g-groups per W chunk
            for ck in range(NG // GCH):    # 6 W chunks, each reused by 4 q4
                wchunk = wstr.tile([128, GCH*NO], BF16, tag="wchunk")
                nc.sync.dma_start(wchunk[:], wk[:, ck*GCH*NO:(ck+1)*GCH*NO])
                for q4 in range(4):
                    # 24 consecutive routes rb..rb+23 (r-major within block)
                    rb = q4*NG + ck*GCH
                    stgt = stgp.tile([64, GCH*NO], BF16, tag="stgt")
                    for i3 in range(GCH // 3):
                        psr = psumB.tile([64, 3*NO], F32, tag="psr")
                        for j in range(3):
                            i = i3*3 + j
                            g = ck*GCH + i
                            nc.tensor.matmul(
                                psr[:, j*NO:(j+1)*NO],
                                fap(u_all, 32*q4, 32, g, [[NG, B]]),
                                wchunk[32*q4:32*q4+32, i*NO:(i+1)*NO],
                                start=True, stop=True, tile_position=(32*q4, 0))
                        nc.scalar.activation(stgt[:, i3*3*NO:(i3+1)*3*NO],
                                             psr[:], AF.Identity, bias=zt[0:64, :])
                    nc.sync.dma_start(
                        ap(uh_dram, rb*B*NO,
                           [[NO, B], [B*NO, GCH], [1, NO]]),
                        stgt[:])
            tc.strict_bb_all_engine_barrier()
            for t in range(NT):
                rows = 128 if t < 4 else 64
                nc.sync.dma_start(
                    uh[t][0:rows, :],
                    ap(uh_dram, t*128*B*NO, [[B*NO, rows], [1, B*NO]]))
                if debug:
                    nc.sync.dma_start(
                        ap(dbg['uh_dbg'], t*128*B*NO, [[B*NO, rows], [1, B*NO]]),
                        uh[t][0:rows, :])

        # ============ routing ============
        rp = top.enter_context(tc.tile_pool(name="rp", bufs=1))
        logits = rp.tile([128, NT*B*ND], F32)
        nc.vector.memset(logits[:], 0.0)
        c_t = rp.tile([128, NT*B*ND], BF16)
        s_sb = rp.tile([10, B*DC], F32)
        v_sb = rp.tile([10, B*DC], F32)
        vb16 = rp.tile([10, B*DC], BF16)
        sq2 = rp.tile([10, B*DC], F32)
        snv = rp.tile([10, B], F32)
        snv1 = rp.tile([10, B], F32)
        rdev = rp.tile([10, B], F32)
        sqv = rp.tile([10, B], F32)
        rsqv = rp.tile([10, B], F32)
        sclv = rp.tile([10, B], F32)
        sume = rp.tile([128, NT*B], F32)
        rece = rp.tile([128, NT*B], F32)

        with tc.tile_pool(name="agg", bufs=1) as agg, \
             tc.tile_pool(name="m2cp", bufs=3) as m2cp, \
             tc.tile_pool(name="sstg", bufs=4) as sstg, \
             tc.tile_pool(name="psumR", bufs=4, space="PSUM") as psumR:
            m2b = agg.tile([128, B*NO//2], BF16)
            red = agg.tile([128, B*ND], F32)
            v_rep = agg.tile([128, B*NO], BF16)
            vrow = agg.tile([1, B*NO], BF16)

            for it in range(3):
                # ---- partial s = sum over local routes of c * u_hat ----
                for chk in range(22):
                    b0 = chk*3
                    nb = min(3, B - b0)
                    w = nb*NO
                    pss = psumR.tile([128, 512], F32, tag="pss")
                    for t in range(NT):
                        kk = 128 if t < 4 else 64
                        if it == 0:
                            rhs = uh[t][0:kk, b0*NO:b0*NO + w]
                        else:
                            m2c = m2cp.tile([128, 512], BF16, tag="m2c")
                            nc.vector.tensor_mul(
                                m2c[0:kk, 0:w], uh[t][0:kk, b0*NO:b0*NO + w],
                                fap(c_t, 0, kk, t*B*ND + b0*ND,
                                    [[ND, nb], [1, ND], [0, DC]]))
                            rhs = m2c[0:kk, 0:w]
                        nc.tensor.matmul(pss[0:1, 0:w], onesb[0:kk, :], rhs,
                                         start=(t == 0), stop=(t == 4))
                    sst = sstg.tile([1, 512], F32, tag="sst")
                    nc.scalar.activation(sst[0:1, 0:w], pss[0:1, 0:w],
                                         AF.Identity, bias=zt[0:1, :],
                                         scale=(0.1 if it == 0 else 1.0))
                    nc.sync.dma_start(ap(s_send, it*B*NO + b0*NO, [[1, w]]),
                                      sst[0:1, 0:w])
                nc.gpsimd.collective_compute(
                    "AllReduce", ALU.add, replica_groups=groups,
                    ins=[ap(s_send, it*B*NO, [[1, B*NO]])],
                    outs=[ap(s_recv, it*B*NO, [[1, B*NO]])])

                # ---- v = squash(s) in [10p=n, (b, o)] ----
                nc.sync.dma_start(s_sb[:],
                                  ap(s_recv, it*B*NO, [[DC, ND], [NO, B], [1, DC]]))
                nc.vector.tensor_mul(sq2[:], s_sb[:], s_sb[:])
                nc.vector.tensor_reduce(snv[:], fap(sq2, 0, 10, 0, [[DC, B], [1, DC]]),
                                        AX.X, ALU.add)
                nc.vector.tensor_scalar_add(snv1[:], snv[:], 1.0)
                nc.vector.reciprocal(rdev[:], snv1[:])
                nc.scalar.activation(sqv[:], snv[:], AF.Sqrt, bias=epst[0:10, :])
                nc.vector.reciprocal(rsqv[:], sqv[:])
                nc.vector.tensor_mul(sclv[:], snv[:], rdev[:])
                nc.vector.tensor_mul(sclv[:], sclv[:], rsqv[:])
                nc.vector.tensor_mul(v_sb[:], s_sb[:],
                                     fap(sclv, 0, 10, 0, [[1, B], [0, DC]]))
                if it == 2:
                    nc.sync.dma_start(v_stage[:].bitcast(F32), v_sb[:])
                    break
                nc.vector.tensor_copy(vb16[:], v_sb[:])
                # vrow [1, (b,n,o)] then broadcast to 128 partitions via PE
                for n in range(ND):
                    nc.sync.dma_start(
                        fap(vrow, 0, 1, n*DC, [[NO, B], [1, DC]]),
                        vb16[n:n+1, :])
                for k in range(B*NO//512):
                    psv = psumR.tile([128, 512], F32, tag="psv")
                    nc.tensor.matmul(psv[:], onesrow[0:1, :],
                                     vrow[0:1, k*512:(k+1)*512],
                                     start=True, stop=True)
                    nc.scalar.activation(v_rep[:, k*512:(k+1)*512], psv[:],
                                         AF.Identity, bias=zt[:, :])

                # ---- agreement: logits += sum_o u_hat * v (half-tiles) ----
                for t in range(NT):
                    for hh in range(2):
                        sl = slice(hh*B*NO//2, (hh+1)*B*NO//2)
                        nc.vector.tensor_mul(m2b[:], uh[t][:, sl], v_rep[:, sl])
                        nc.vector.tensor_reduce(
                            red[:, hh*B*ND//2:(hh+1)*B*ND//2],
                            fap(m2b, 0, 128, 0, [[DC, B*ND//2], [1, DC]]),
                            AX.X, ALU.add)
                    nc.vector.tensor_add(
                        logits[:, t*B*ND:(t+1)*B*ND],
                        logits[:, t*B*ND:(t+1)*B*ND],
                        red[:])

                # ---- c = softmax(logits) over n (no max-subtraction) ----
                eexp = v_rep[:].bitcast(F32)        # [128, 5120] f32 view
                nc.scalar.activation(eexp[:, 0:NT*B*ND], logits[:], AF.Exp, bias=zt[:, :])
                nc.vector.tensor_reduce(
                    sume[:], fap(eexp, 0, 128, 0, [[ND, NT*B], [1, ND]]),
                    AX.X, ALU.add)
                nc.vector.reciprocal(rece[:], sume[:])
                nc.vector.tensor_mul(c_t[:], eexp[:, 0:NT*B*ND],
                                     fap(rece, 0, 128, 0, [[1, NT*B], [0, ND]]))

        # ============ FC head (redundant on every core) ============
        with tc.tile_pool(name="fcp", bufs=1) as fcp, \
             tc.tile_pool(name="psumF", bufs=1, space="PSUM") as psumF:
            fta = fcp.tile([128, B], F32R)
            ftb = fcp.tile([128, B], F32R)
            for n in range(ND):
                dstt, p0 = (fta, n*DC) if n < 8 else (ftb, (n-8)*DC)
                nc.sync.dma_start(dstt[p0:p0+DC, :],
                                  ap(v_stage, n*B*DC, [[1, DC], [DC, B]]))
            nc.sync.dma_start(ftb[32:33, :], onesd[0:1, :])

            fc1a = fcp.tile([128, 512], F32R)
            nc.sync.dma_start(fc1a[:], fc1t[0:128, :])
            fc1b = fcp.tile([128, 512], F32R)
            nc.sync.dma_start(fc1b[0:33, :], fc1t[128:161, :])
            pf1 = psumF.tile([64, 512], F32, tag="pf1")
            nc.tensor.matmul(pf1[:], fta[:, 0:64],
                             fc1a[:], start=True, stop=False)
            nc.tensor.matmul(pf1[:], ftb[0:33, 0:64],
                             fc1b[0:33, :], start=False, stop=True)
            f1 = fcp.tile([64, 512], F32R)
            nc.scalar.activation(f1[:], pf1[:], AF.Relu, bias=zt[0:64, :])

            f1T = fcp.tile([128, 4*64], F32R)
            for k in range(4):
                ptr = psumF.tile([128, 64], F32R, tag="ptr", bufs=2)
                nc.tensor.transpose(ptr[:], f1[:, k*128:(k+1)*128], identT[0:64, 0:64])
                nc.scalar.activation(f1T[:, k*64:(k+1)*64], ptr[:], AF.Identity, bias=zt[:, :])

            fc2a = fcp.tile([128, 4*256], F32R)
            nc.sync.dma_start(fc2a[:], ap(fc2t, 0, [[256, 128], [128*256, 4], [1, 256]]))
            fc2b = fcp.tile([1, 256], F32R)
            nc.sync.dma_start(fc2b[:], fc2t[512:513, :])
            pf2 = psumF.tile([64, 256], F32, tag="pf2")
            for k in range(4):
                nc.tensor.matmul(pf2[:], f1T[:, k*64:(k+1)*64],
                                 fc2a[:, k*256:(k+1)*256],
                                 start=(k == 0), stop=False)
            nc.tensor.matmul(pf2[:], onesf[0:1, :],
                             fc2b[:], start=False, stop=True)
            f2 = fcp.tile([64, 256], F32R)
            nc.scalar.activation(f2[:], pf2[:], AF.Relu, bias=zt[0:64, :])

            f2T = fcp.tile([128, 2*64], F32R)
            for k in range(2):
                ptr2 = psumF.tile([128, 64], F32R, tag="ptr", bufs=2)
                nc.tensor.transpose(ptr2[:], f2[:, k*128:(k+1)*128], identT[0:64, 0:64])
                nc.scalar.activation(f2T[:, k*64:(k+1)*64], ptr2[:], AF.Identity, bias=zt[:, :])

            fc3a = fcp.tile([128, 2*128], F32R)
            nc.sync.dma_start(fc3a[:], ap(fc3t, 0, [[128, 128], [128*128, 2], [1, 128]]))
            fc3b = fcp.tile([1, 128], F32R)
            nc.sync.dma_start(fc3b[:], fc3t[256:257, :])
            pf3 = psumF.tile([64, 128], F32, tag="pf3")
            for k in range(2):
                nc.tensor.matmul(pf3[:], f2T[:, k*64:(k+1)*64],
                                 fc3a[:, k*128:(k+1)*128],
                                 start=(k == 0), stop=False)
            nc.tensor.matmul(pf3[:], onesf[0:1, :],
                             fc3b[:], start=False, stop=True)
            fout = fcp.tile([64, 128], F32)
            nc.scalar.activation(fout[:], pf3[:], AF.Identity, bias=zt[0:64, :])
            nc.sync.dma_start(out[:], fout[:])

    nc.compile()
    return nc


# ---------------------------------------------------------------------------
# host side
# ---------------------------------------------------------------------------
def _bf16(x):
    import ml_dtypes
    return np.asarray(x, np.float32).astype(ml_dtypes.bfloat16)


def prep_inputs(x, conv1_w, conv1_b, bn_g, bn_b, pconv_w, pconv_b, W_caps,
                fc1_w, fc1_b, fc2_w, fc2_b, fc3_w, fc3_b):
    x = np.asarray(x, np.float32)
    s_bn = (np.asarray(bn_g) / np.sqrt(1.0 + 1e-5)).astype(np.float32)
    w1f = (np.asarray(conv1_w) * s_bn[:, None, None, None]).astype(np.float32)
    b1v = (np.asarray(conv1_b)*s_bn + np.asarray(bn_b)).astype(np.float32)
    w1m = np.ascontiguousarray(np.transpose(w1f, (1, 2, 3, 0)).reshape(243, 256))
    w2t = np.transpose(np.asarray(pconv_w, np.float32), (1, 2, 3, 0))  # [ic,ky,kx,oc]
    w2t = w2t.reshape(2, 128, 9, 9, 2, 128)
    w2m = _bf16(np.ascontiguousarray(
        np.transpose(w2t, (1, 0, 2, 3, 4, 5)).reshape(128, 2*81*2*128)))
    W_caps = np.asarray(W_caps, np.float32)
    wks = []
    for k in range(N_CORES):
        Wk = W_caps[RPC*k:RPC*(k+1)]            # [576, 10, 32, 16]
        # r = q4*144 + g ; partition = q4*32 + c ; col = g*160 + n*16 + o
        wkm = np.transpose(Wk.reshape(4, NG, ND, PC, DC), (0, 3, 1, 2, 4))
        wks.append(_bf16(np.ascontiguousarray(wkm.reshape(128, NG*ND*DC))))
    fc1m = np.concatenate([np.asarray(fc1_w).T, np.asarray(fc1_b)[None, :]], 0).astype(np.float32)
    fc2m = np.concatenate([np.asarray(fc2_w).T, np.asarray(fc2_b)[None, :]], 0).astype(np.float32)
    fc3m = np.concatenate([np.asarray(fc3_w).T, np.asarray(fc3_b)[None, :]], 0).astype(np.float32)
    identm = np.eye(128, dtype=np.float32)
    in_maps = []
    for k in range(N_CORES):
        in_maps.append({
            "xs": np.ascontiguousarray(x[BPC*k:BPC*(k+1)]),
            "w1": w1m, "b1": b1v, "w2": w2m,
            "b2": np.asarray(pconv_b, np.float32),
            "wk": wks[k],
            "fc1t": fc1m, "fc2t": fc2m, "fc3t": fc3m,
            "ident": identm, "onesd": np.ones((128, 64), np.float32),
        })
    return in_maps


_NC_CACHE = {}


def kernel(**inputs):
    if 'main' not in _NC_CACHE:
        # debug=True keeps the intermediate dump DMAs; they act as pipeline
        # spacers that enforce the A2A/transpose/DRAM-stage orderings the
        # scheduler otherwise misses (NaN without them), at ~0.1% HW cost.
        _NC_CACHE['main'] = build(debug=(os.environ.get('KDBG','1')=='1'),
                                  declare_dbg=(os.environ.get('KDECL','0')=='1') or None)
    nc = _NC_CACHE['main']
    in_maps = prep_inputs(**{k: np.asarray(v) for k, v in inputs.items()})
    res = run_bass_kernel_spmd(nc, in_maps, list(range(N_CORES)))
    return np.asarray(res.results[0]["out"], dtype=np.float32)
